# revision 1
# baseline (speedup 1.0000x reference)
"""Trainium2 Bass kernel for DAN embedding-bag + linear head.

Computes out = (1/rowsum(x)) * (x @ embeds) @ fc_w.T + fc_b for
x [8192, 12820] f32 by algebraically collapsing the two matmuls:
    out[:, e] = (x @ (embeds @ fc_w.T + b)[:, e]) / (x @ ones)
The [12820, 2] collapsed weight is computed on the host; the device
kernel is a pure memory-bound streaming reduction over x (400 MB),
data-parallel across 8 NeuronCores (1024 rows each).

Per-core pipeline (overlapped; DMA measured at the 149 us/pass HBM
roofline, full kernel at ~295 us/pass by repetition-slope timing):
  sync-DMA   x f32 chunks [128, 6410] -> SBUF (352 GB/s/core)
  ScalarE    copy f32->bf16 with fused accum_out = f32 row-sum
  VectorE    tensor_mul x_bf16 * w_col at 2x bf16 mode (products bf16);
             w replicated across partitions once via on-chip spread
  reduce     free-dim sum of each product: split 19/13 between ScalarE
             (activation+accum_out) and VectorE (tensor_reduce, 1x)
  VectorE    epilogue: reciprocal + scale, one [128, 16] tile
  sync-DMA   out [1024, 2]

Measured op costs that drove this design (trn2, [128, 6410] bf16):
  tensor_tensor mult 2x ~3.4us (hides under DMA); tensor_scalar or
  tensor_reduce with accum 1x ~6.7us; scalar_tensor_tensor fused
  multiply+accum ~21us (avoid); tensor_tensor_reduce: not supported
  by this neuronxcc. ACT activation+accum ~5.5us, overlaps well.
"""

import sys

if "/opt/trn_rl_repo" not in sys.path:
    sys.path.insert(0, "/opt/trn_rl_repo")

import json

import ml_dtypes
import numpy as np

import concourse.bass as bass
import concourse.mybir as mybir
from concourse import tile
from concourse.bass_utils import run_bass_kernel_spmd

N_CORES = 8
N = 8192
K = 12820
EMB = 320
ROWS = N // N_CORES  # 1024 rows per core
P = 128
M_TILES = ROWS // P  # 8
N_CHUNKS = 2
FD = K // N_CHUNKS  # 6410
WREP = 16  # partitions of pre-replicated w shipped from host

BF16 = ml_dtypes.bfloat16

# ---------------------------------------------------------------------------
# The neuronxcc walrus in this container rejects any instruction carrying
# more than one sync-wait command. TileContext can emit several (drain,
# multi-dep consumers). Split extras onto preceding NoOps on the same
# engine at BIR-JSON serialization time.
_MAX_WAITS = 1
_wait_split_installed = False


def _split_multi_waits(bir: dict) -> dict:
    ctr = 0
    for fn in bir.get("functions", []):
        for blk in fn.get("blocks", []):
            new_insts = []
            for inst in blk.get("instructions", []):
                si = inst.get("sync_info")
                waits = si.get("on_wait") if si else None
                if waits and len(waits) > _MAX_WAITS:
                    extra = waits[: -_MAX_WAITS]
                    si["on_wait"] = waits[-_MAX_WAITS:]
                    for j in range(0, len(extra), _MAX_WAITS):
                        ctr += 1
                        new_insts.append(
                            {
                                "debug": inst.get("debug", 0),
                                "engine": inst["engine"],
                                "ins": [],
                                "outs": [],
                                "name": f"I-wsplit-{ctr}",
                                "opcode": "NoOp",
                                "sync_info": {
                                    "on_update": [],
                                    "on_wait": extra[j : j + _MAX_WAITS],
                                },
                            }
                        )
                new_insts.append(inst)
            blk["instructions"] = new_insts
    return bir


def _install_wait_split():
    global _wait_split_installed
    if _wait_split_installed:
        return
    orig = bass.Bass.to_json_bytes

    def patched(self):
        d = json.loads(orig(self))
        _split_multi_waits(d)
        return json.dumps(d).encode()

    bass.Bass.to_json_bytes = patched
    _wait_split_installed = True


# ---------------------------------------------------------------------------


def build_bass(reps: int = 1, stages: str = "full2", n_chunks: int = N_CHUNKS):
    """Build the per-core Bass program (identical on all 8 cores).

    reps>1 unrolls the whole body for slope-based timing; stages in
    {"dma", "act", "full", "full2", "tt", "ts", "stt1"} picks variants
    for bottleneck decomposition (tt/ts/stt1 compute wrong results —
    timing only). kernel() always uses reps=1, stages="full2".
    """
    _install_wait_split()
    nc = bass.Bass(
        "TRN2", target_bir_lowering=False, debug=False, num_devices=N_CORES
    )
    x_in = nc.dram_tensor(
        "x", [ROWS, K], mybir.dt.float32, kind="ExternalInput"
    ).ap()
    w_in = nc.dram_tensor(
        "w", [WREP, 2 * K], mybir.dt.bfloat16, kind="ExternalInput"
    ).ap()
    y_out = nc.dram_tensor(
        "y", [ROWS, 2], mybir.dt.float32, kind="ExternalOutput"
    ).ap()

    f32 = mybir.dt.float32
    bf16 = mybir.dt.bfloat16
    mult = mybir.AluOpType.mult
    Copy = mybir.ActivationFunctionType.Copy

    n_act_reduce = 19  # of 32 chunk-col reduces, how many ride on ScalarE

    with tile.TileContext(nc) as tc:
        with (
            tc.tile_pool(name="wpool", bufs=1) as wpool,
            tc.tile_pool(name="xf", bufs=4 if n_chunks >= 4 else 3) as xfpool,
            tc.tile_pool(name="xb", bufs=4 if n_chunks >= 4 else 2) as xbpool,
            tc.tile_pool(name="prod", bufs=4 if n_chunks >= 4 else 2) as ppool,
            tc.tile_pool(name="scratch", bufs=1) as spool,
            tc.tile_pool(name="acc", bufs=1) as apool,
        ):
            # --- replicated weights: load 16 partitions, spread to 128 ---
            w_sb = wpool.tile([P, 2 * K], bf16)
            nc.sync.dma_start(out=w_sb[0:WREP, :], in_=w_in[:, :])
            for g in range(1, P // WREP):
                nc.sync.dma_start(
                    out=w_sb[g * WREP : (g + 1) * WREP, :], in_=w_sb[0:WREP, :]
                )

            # --- accumulator slabs: slot index = m * N_CHUNKS + c ---
            nslot = M_TILES * N_CHUNKS
            acc0 = apool.tile([P, nslot], f32, tag="acc0")
            acc1 = apool.tile([P, nslot], f32, tag="acc1")
            acc2 = apool.tile([P, nslot], f32, tag="acc2")

            fd = K // n_chunks
            scratch = spool.tile([P, fd], bf16)

            for _rep in range(reps):
                for m in range(M_TILES):
                    for c in range(n_chunks):
                        slot = (m * n_chunks + c) % nslot
                        xf = xfpool.tile([P, fd], f32)
                        nc.sync.dma_start(
                            out=xf[:, :],
                            in_=x_in[m * P : (m + 1) * P, c * fd : (c + 1) * fd],
                        )
                        if stages == "dma":
                            continue
                        xb = xbpool.tile([P, fd], bf16)
                        # downcast + fused f32 row-sum
                        nc.scalar.activation(
                            out=xb[:, :],
                            in_=xf[:, :],
                            func=Copy,
                            accum_out=acc2[:, slot : slot + 1],
                        )
                        if stages == "act":
                            continue
                        if stages == "tt":
                            nc.vector.tensor_mul(
                                scratch[:, :], xb[:, :], w_sb[:, c * fd : (c + 1) * fd]
                            )
                            nc.vector.tensor_mul(
                                scratch[:, :], xb[:, :], w_sb[:, c * fd : (c + 1) * fd]
                            )
                            continue
                        if stages == "ts":
                            add = mybir.AluOpType.add
                            nc.vector.tensor_scalar(
                                scratch[:, :], xb[:, :], 2.0, 0.0,
                                op0=mult, op1=add,
                                accum_out=acc0[:, slot : slot + 1],
                            )
                            nc.vector.tensor_scalar(
                                scratch[:, :], xb[:, :], 2.0, 0.0,
                                op0=mult, op1=add,
                                accum_out=acc1[:, slot : slot + 1],
                            )
                            continue
                        if stages == "ttr":
                            add = mybir.AluOpType.add
                            nc.vector.tensor_tensor_reduce(
                                out=scratch[:, :], in0=xb[:, :],
                                in1=w_sb[:, c * fd : (c + 1) * fd],
                                scale=1.0, scalar=0.0, op0=mult, op1=add,
                                accum_out=acc0[:, slot : slot + 1],
                            )
                            nc.vector.tensor_tensor_reduce(
                                out=scratch[:, :], in0=xb[:, :],
                                in1=w_sb[:, K + c * fd : K + (c + 1) * fd],
                                scale=1.0, scalar=0.0, op0=mult, op1=add,
                                accum_out=acc1[:, slot : slot + 1],
                            )
                            continue
                        if stages == "full2":
                            # TT-mult at 2x, then reduce on DVE or ACT
                            for col, accx in ((0, acc0), (1, acc1)):
                                prod = ppool.tile([P, fd], bf16, tag="prod")
                                nc.vector.tensor_mul(
                                    prod[:, :],
                                    xb[:, :],
                                    w_sb[:, col * K + c * fd : col * K + (c + 1) * fd],
                                )
                                idx = (m * n_chunks + c) * 2 + col
                                if idx % 32 < n_act_reduce:
                                    nc.scalar.activation(
                                        out=scratch[:, :],
                                        in_=prod[:, :],
                                        func=Copy,
                                        accum_out=accx[:, slot : slot + 1],
                                    )
                                else:
                                    nc.vector.tensor_reduce(
                                        accx[:, slot : slot + 1],
                                        prod[:, :],
                                        axis=mybir.AxisListType.X,
                                        op=mybir.AluOpType.add,
                                    )
                            continue
                        # fused multiply + free-dim sum, per output column
                        nc.vector.scalar_tensor_tensor(
                            out=scratch[:, :],
                            in0=xb[:, :],
                            scalar=1.0,
                            in1=w_sb[:, c * fd : (c + 1) * fd],
                            op0=mult,
                            op1=mult,
                            accum_out=acc0[:, slot : slot + 1],
                        )
                        if stages == "stt1":
                            continue
                        nc.vector.scalar_tensor_tensor(
                            out=scratch[:, :],
                            in0=xb[:, :],
                            scalar=1.0,
                            in1=w_sb[:, K + c * fd : K + (c + 1) * fd],
                            op0=mult,
                            op1=mult,
                            accum_out=acc1[:, slot : slot + 1],
                        )

                # --- epilogue: combine chunk partials, divide, store ---
                tot0 = apool.tile([P, M_TILES], f32, tag="tot0")
                tot1 = apool.tile([P, M_TILES], f32, tag="tot1")
                tot2 = apool.tile([P, M_TILES], f32, tag="tot2")
                rcp = apool.tile([P, M_TILES], f32, tag="rcp")
                outt = apool.tile([P, M_TILES * 2], f32, tag="outt")

                if stages in ("full", "full2"):
                    nc.vector.tensor_add(
                        tot0[:, :], acc0[:, 0 : nslot : 2], acc0[:, 1 : nslot : 2]
                    )
                    nc.vector.tensor_add(
                        tot1[:, :], acc1[:, 0 : nslot : 2], acc1[:, 1 : nslot : 2]
                    )
                    nc.vector.tensor_add(
                        tot2[:, :], acc2[:, 0 : nslot : 2], acc2[:, 1 : nslot : 2]
                    )
                    nc.vector.reciprocal(rcp[:, :], tot2[:, :])
                    nc.vector.tensor_mul(
                        outt[:, 0 : 2 * M_TILES : 2], tot0[:, :], rcp[:, :]
                    )
                    nc.vector.tensor_mul(
                        outt[:, 1 : 2 * M_TILES : 2], tot1[:, :], rcp[:, :]
                    )
                else:
                    nc.vector.tensor_scalar_mul(outt[:, :], outt[:, :], 0.0)

                # y[m*128 + p, e] = outt[p, 2*m + e]
                y_view = y_out.rearrange("(m p) e -> p m e", p=P)
                nc.sync.dma_start(out=y_view, in_=outt[:, :])

    return nc


def host_weights(embeds: np.ndarray, fc_w: np.ndarray, fc_b: np.ndarray):
    """Collapse embeds/fc into the [WREP, 2K] bf16 device weight."""
    w2 = embeds.astype(np.float32) @ fc_w.astype(np.float32).T  # [K, 2]
    w2 = w2 + fc_b.astype(np.float32)[None, :]  # fold bias
    flat = np.concatenate([w2[:, 0], w2[:, 1]]).astype(BF16)  # [2K]
    return np.tile(flat[None, :], (WREP, 1))  # [WREP, 2K]


_NC_CACHE = None


def get_nc():
    global _NC_CACHE
    if _NC_CACHE is None:
        _NC_CACHE = build_bass()
    return _NC_CACHE


def make_in_maps(x: np.ndarray, w_rep: np.ndarray):
    x = np.ascontiguousarray(x, dtype=np.float32)
    return [
        {"x": x[i * ROWS : (i + 1) * ROWS], "w": w_rep} for i in range(N_CORES)
    ]


def kernel(x, embeds, fc_w, fc_b):
    x = np.asarray(x, dtype=np.float32)
    w_rep = host_weights(np.asarray(embeds), np.asarray(fc_w), np.asarray(fc_b))
    nc = get_nc()
    res = run_bass_kernel_spmd(
        nc, make_in_maps(x, w_rep), core_ids=list(range(N_CORES))
    )
    return np.concatenate(
        [res.results[i]["y"] for i in range(N_CORES)], axis=0
    ).astype(np.float32)

